# revision 22
# baseline (speedup 1.0000x reference)
"""Trainium2 Bass kernel for LocalDenseSynthesizerAttention (band C=63, H=4 heads).

Sharding: 8192 tokens (B=2 x T=4096 flattened) split contiguously across 8
cores (1024 tokens each).  Each core runs an identical program on its own
slice; batch-edge band masking and value halo padding are handled host-side
via per-core input data (masks / zero-padded valueT), so the program is
uniform SPMD.

Band pipeline, processed in two 4-tile halves: scores -> exp -> batched
softmax-normalize (one reduce/recip/multiply per half on DVE) -> ONE
diagonal-scatter DMA per half (Pool/SWDGE) into a host-zeroed DRAM stage
holding the token-major banded matrix S for all 4 tiles (row pitch 4096+1)
-> per-tile XBAR DMA-transposes (row step 4096; the pitch mismatch forms
the band) producing S^T chunks in SBUF -> band matmuls against parked V.
The DRAM stage is an ExternalInput pre-zeroed by the host, so off-band
zeros cost no device time; no PE transposes or staging copies are needed.
"""

import numpy as np
import ml_dtypes

import concourse.bass as bass
import concourse.bacc as bacc
import concourse.mybir as mybir
import concourse.tile as tile
from concourse.ap import AP
from concourse import bass_utils

BF16 = mybir.dt.bfloat16
FP32 = mybir.dt.float32
NP_BF16 = ml_dtypes.bfloat16

B, T, NF = 2, 4096, 256
H, C, DK = 4, 63, 64
HALF = (C - 1) // 2  # 31
N_CORES = 8
TPC = (B * T) // N_CORES  # 1024 tokens per core
N_TILES = TPC // 128  # 8
VPAD = 1152  # parked value rows: tokens [-31, 1121) relative to core start
SWH = 256  # per-head stage width (window rows 0..189, pad to 256)
TPH = 4  # tiles per half
ROWP = TPH * H * SWH  # 4096 virtual row size of a half-stage
STG_ELEMS = 128 * ROWP + 192  # covers write (pitch ROWP+1) and read (ROWP)


def build_program(reps: int = 1):
    import contextlib

    nc = bacc.Bacc(
        "TRN2",
        target_bir_lowering=False,
        debug=False,
        enable_asserts=False,
        num_devices=N_CORES,
    )

    # DRAM I/O (per-core data, same names on every core)
    qT_d = nc.dram_tensor("qT", [NF, TPC], BF16, kind="ExternalInput").ap()
    vT_d = nc.dram_tensor("vT", [NF, VPAD], BF16, kind="ExternalInput").ap()
    wpack_d = nc.dram_tensor("wpack", [NF, 1020], BF16, kind="ExternalInput").ap()
    maskp_d = nc.dram_tensor("maskp", [128, 2 * C], FP32, kind="ExternalInput").ap()
    # host-zeroed band stages (one per 4-tile half); off-band stays zero
    stg_d = [
        nc.dram_tensor(f"stg{i}", [STG_ELEMS], BF16, kind="ExternalInput").ap()
        for i in range(2)
    ]
    outT_d = nc.dram_tensor("outT", [NF, TPC], BF16, kind="ExternalOutput").ap()

    with tile.TileContext(nc) as tc:
        with (
            tc.tile_pool(name="inp", bufs=1) as inp,
            tc.tile_pool(name="hbuf", bufs=2) as hbuf,
            tc.tile_pool(name="sta_p", bufs=8) as sta_p,
            tc.tile_pool(name="big_ps", bufs=2, space="PSUM") as big_ps,
            tc.tile_pool(name="sc_ps", bufs=2, space="PSUM") as sc_ps,
            tc.tile_pool(name="ob_ps", bufs=2, space="PSUM") as ob_ps,
            tc.tile_pool(name="x_ps", bufs=2, space="PSUM") as x_ps,
        ):
            # ---- persistent SBUF tensors --------------------------------
            qt_in = inp.tile([128, 2, TPC], BF16, tag="qt_in")
            vt_in = inp.tile([128, 2, VPAD], BF16, tag="vt_in")
            wall = inp.tile([128, 2, 1020], BF16, tag="wall")
            maskp = inp.tile([128, 2 * C], FP32, tag="maskp")
            w1t = wall[:, :, 0:256]
            w2t = wall[:, :, 256:508]
            w3t = wall[:, :, 508:764]
            wot = wall[:, :, 764:1020]
            qtr = inp.tile([128, 2, TPC], BF16, tag="qtr")
            vpark = inp.tile([128, 9, NF], BF16, tag="vpark")
            xt = inp.tile([128, 2, TPC], BF16, tag="xt")
            outsb = inp.tile([128, 2, TPC], BF16, tag="outsb")

            wpack_r = wpack_d.rearrange("(c p) t -> p c t", p=128)
            qT_r = qT_d.rearrange("(c p) t -> p c t", p=128)
            vT_r = vT_d.rearrange("(c p) t -> p c t", p=128)
            outT_r = outT_d.rearrange("(c p) t -> p c t", p=128)

            loop_ctx = (
                tc.For_i(0, reps, 1, hint_engines=(mybir.EngineType.PE,))
                if reps > 1
                else contextlib.nullcontext()
            )
            with loop_ctx:
                # ---- input DMAs: q + w1 first so stage 1 starts ASAP ---
                nc.sync.dma_start(qt_in[:, :, 0:512], qT_r[:, :, 0:512])
                nc.sync.dma_start(wall[:, :, 0:256], wpack_r[:, :, 0:256])
                nc.sync.dma_start(qt_in[:, :, 512:TPC], qT_r[:, :, 512:TPC])
                nc.sync.dma_start(wall[:, :, 256:1020], wpack_r[:, :, 256:1020])
                nc.sync.dma_start(vt_in[:], vT_r)
                nc.scalar.dma_start(maskp[:], maskp_d)

                # ---- stage 1: qTr = relu(w1 @ queryT) ------------------
                for m in range(2):  # mega-tiles of 512 tokens
                    for mc in range(2):  # output feature chunk
                        ps = big_ps.tile([128, 512], FP32, tag="big")
                        for kc in range(2):
                            nc.tensor.matmul(
                                ps[:],
                                w1t[:, kc, mc * 128 : (mc + 1) * 128],
                                qt_in[:, kc, m * 512 : (m + 1) * 512],
                                start=(kc == 0),
                                stop=(kc == 1),
                            )
                        nc.scalar.activation(
                            qtr[:, mc, m * 512 : (m + 1) * 512],
                            ps[:],
                            mybir.ActivationFunctionType.Relu,
                        )

                # ---- band pipeline ------------------------------------
                sta_tiles = [None] * (N_TILES // 2)

                def emit_vproj():
                    # stage 2: V = value @ w3.T parked at -31 offset
                    for vp in range(5):  # pairs of V tiles share a PSUM bank
                        nv = 2 if vp < 4 else 1
                        ps = big_ps.tile([128, 512], FP32, tag="big")
                        for j in range(nv):
                            vt = 2 * vp + j
                            for kc in range(2):
                                nc.tensor.matmul(
                                    ps[:, j * 256 : (j + 1) * 256],
                                    vt_in[:, kc, vt * 128 : (vt + 1) * 128],
                                    w3t[:, kc, :],
                                    start=(kc == 0),
                                    stop=(kc == 1),
                                )
                        dst = vpark[:, 2 * vp : 2 * vp + nv, :]
                        src = ps[:, 0 : nv * 256].rearrange("p (a b) -> p a b", a=nv)
                        if vp in (0, 2, 4):
                            nc.vector.tensor_copy(dst, src)
                        else:
                            nc.scalar.activation(
                                dst, src, mybir.ActivationFunctionType.Copy
                            )

                def emit_passA(hf):
                    """Scores -> exp -> per-tile softmax chain on DVE, then
                    ONE diagonal scatter for the 4-tile half."""
                    pnh = hbuf.tile([128, TPH, H * C], BF16, tag="pnh")
                    for j in range(TPH):
                        t = TPH * hf + j
                        sc = sc_ps.tile([128, H * C], FP32, tag="sc")
                        for kc in range(2):
                            nc.tensor.matmul(
                                sc[:],
                                qtr[:, kc, t * 128 : (t + 1) * 128],
                                w2t[:, kc, :],
                                start=(kc == 0),
                                stop=(kc == 1),
                            )
                        if t == 0 or t == N_TILES - 1:
                            moff = 0 if t == 0 else C
                            mask_ap = AP(
                                maskp[:].tensor,
                                maskp[:].offset + moff,
                                [[2 * C, 128], [0, H], [1, C]],
                            )
                            nc.vector.tensor_add(
                                sc[:].rearrange("p (h c) -> p h c", h=H),
                                sc[:].rearrange("p (h c) -> p h c", h=H),
                                mask_ap,
                            )
                        expp = hbuf.tile([128, H * C], BF16, tag="expp")
                        nc.scalar.activation(
                            expp[:], sc[:], mybir.ActivationFunctionType.Exp
                        )
                        den = hbuf.tile([128, H], FP32, tag="den")
                        nc.vector.tensor_reduce(
                            den[:],
                            expp[:].rearrange("p (h c) -> p h c", h=H),
                            axis=mybir.AxisListType.X,
                            op=mybir.AluOpType.add,
                        )
                        rden = hbuf.tile([128, H], FP32, tag="rden")
                        nc.vector.reciprocal(rden[:], den[:])
                        rb = AP(
                            rden[:].tensor, rden[:].offset, [[H, 128], [1, H], [0, C]]
                        )
                        nc.vector.tensor_mul(
                            pnh[:, j, :].rearrange("p (h c) -> p h c", h=H),
                            expp[:].rearrange("p (h c) -> p h c", h=H),
                            rb,
                        )
                    # ONE diagonal scatter for the half:
                    # stg[i*(ROWP+1) + k*SWH + c] = pnh[i, k//H, (k%H)*C + c]
                    stg = stg_d[hf]
                    diag_dst = AP(
                        stg.tensor,
                        stg.offset,
                        [[ROWP + 1, 128], [SWH, TPH * H], [1, C]],
                    )
                    nc.sync.dma_start(
                        diag_dst,
                        pnh[:].rearrange("p j (h c) -> p (j h) c", h=H),
                    )

                def emit_transpose(pr):
                    """XBAR transpose of a 2-tile stage window; reading rows
                    at step ROWP (vs ROWP+1 written) forms the band."""
                    hf, jp = divmod(pr, 2)
                    sta = sta_p.tile([128, 16, 128], BF16, tag="sta")
                    src = AP(
                        stg_d[hf].tensor,
                        stg_d[hf].offset + jp * 2 * H * SWH,
                        [[ROWP, 128], [1, 2 * H * SWH]],
                    )
                    nc.sync.dma_start(sta[:], src, transpose=True)
                    sta_tiles[pr] = sta

                def emit_passB(t):
                    # band matmuls: xT_h = V_ext^T @ S^T (window chunks are
                    # park-tile aligned thanks to the -31 park offset)
                    sta = sta_tiles[t // 2]
                    co = 8 * (t % 2)
                    xps = x_ps.tile([128, 256], FP32, tag="xv")
                    for h in range(H):
                        out_sl = xps[
                            64 * (h % 2) : 64 * (h % 2) + 64,
                            128 * (h // 2) : 128 * (h // 2) + 128,
                        ]
                        nc.tensor.matmul(
                            out_sl,
                            vpark[0:128, t, h * DK : (h + 1) * DK],
                            sta[0:128, co + 2 * h, :],
                            start=True,
                            stop=False,
                        )
                        nc.tensor.matmul(
                            out_sl,
                            vpark[0:62, t + 1, h * DK : (h + 1) * DK],
                            sta[0:62, co + 2 * h + 1, :],
                            start=False,
                            stop=True,
                        )
                    # one copy per tile: (h0,h1 | h2,h3) -> xt feature chunks
                    xdst = AP(
                        xt[:].tensor,
                        xt[:].offset + t * 128,
                        [[2 * TPC, 128], [TPC, 2], [1, 128]],
                    )
                    nc.vector.tensor_copy(
                        xdst, xps[:].rearrange("p (a b) -> p a b", a=2)
                    )

                    # ---- out-projection per 256-token block ------------
                    if t % 2 == 1:
                        m = t // 2
                        for mc in range(2):
                            ps = ob_ps.tile([128, 256], FP32, tag="obig")
                            for kc in range(2):
                                nc.tensor.matmul(
                                    ps[:],
                                    wot[:, kc, mc * 128 : (mc + 1) * 128],
                                    xt[:, kc, m * 256 : (m + 1) * 256],
                                    start=(kc == 0),
                                    stop=(kc == 1),
                                )
                            if mc == 0:
                                nc.vector.tensor_copy(
                                    outsb[:, mc, m * 256 : (m + 1) * 256], ps[:]
                                )
                            else:
                                nc.scalar.activation(
                                    outsb[:, mc, m * 256 : (m + 1) * 256],
                                    ps[:],
                                    mybir.ActivationFunctionType.Copy,
                                )
                        nc.scalar.dma_start(
                            outT_r[:, :, m * 256 : (m + 1) * 256],
                            outsb[:, :, m * 256 : (m + 1) * 256],
                        )

                # interleave so SP's in-order stream is:
                #   inputs, diag0, tr01, tr23, diag1, tr45, tr67
                # while PE runs stage1, scores hf0, vproj, scores hf1, band.
                emit_passA(0)
                emit_transpose(0)
                emit_transpose(1)
                emit_vproj()
                emit_passA(1)
                for t in range(4):
                    emit_passB(t)
                emit_transpose(2)
                emit_transpose(3)
                for t in range(4, 8):
                    emit_passB(t)

    nc.compile()
    return nc


def make_inputs(query, value, w1, w2, w3, w_out):
    """Host-side shard/transpose/cast. Returns per-core in_maps."""
    fq = np.asarray(query, np.float32).reshape(B * T, NF)
    fv = np.asarray(value, np.float32).reshape(B * T, NF)
    wpack = np.zeros((NF, 1020), np.float32)
    wpack[:, 0:256] = np.asarray(w1, np.float32).T
    wpack[:, 256:508] = np.asarray(w2, np.float32).T
    wpack[:, 508:764] = np.asarray(w3, np.float32).T
    wpack[:, 764:1020] = np.asarray(w_out, np.float32).T
    wpack = wpack.astype(NP_BF16)
    stg_zero = np.zeros(STG_ELEMS, NP_BF16)

    in_maps = []
    for c in range(N_CORES):
        t0 = c * TPC
        b = (c * TPC) // T
        b0, b1 = b * T, (b + 1) * T
        qT = np.ascontiguousarray(fq[t0 : t0 + TPC].T).astype(NP_BF16)
        # parked value rows: global tokens [t0-31, t0-31+VPAD), zero outside batch
        vrows = np.zeros((VPAD, NF), np.float32)
        lo = t0 - HALF
        s0, s1 = max(lo, b0), min(lo + VPAD, b1)
        vrows[s0 - lo : s1 - lo] = fv[s0:s1]
        vT = np.ascontiguousarray(vrows.T).astype(NP_BF16)
        # additive band masks for first/last tile (batch edges, head-shared)
        maskp = np.zeros((128, 2 * C), np.float32)
        k = np.arange(C)
        for i in range(128):
            g = t0 + i
            bad = (g + k - HALF < b0) | (g + k - HALF >= b1)
            maskp[i, :C] = np.where(bad, -30000.0, 0.0)
            g = t0 + (N_TILES - 1) * 128 + i
            bad = (g + k - HALF < b0) | (g + k - HALF >= b1)
            maskp[i, C:] = np.where(bad, -30000.0, 0.0)
        im = {"qT": qT, "vT": vT, "wpack": wpack, "maskp": maskp}
        for hf in range(2):
            im[f"stg{hf}"] = stg_zero
        in_maps.append(im)
    return in_maps


_NC_CACHE = None


def kernel(query, key, value, mask, w1, w2, w3, w_out):
    global _NC_CACHE
    if _NC_CACHE is None:
        _NC_CACHE = build_program()
    nc = _NC_CACHE
    in_maps = make_inputs(query, value, w1, w2, w3, w_out)
    res = bass_utils.run_bass_kernel_spmd(nc, in_maps, core_ids=list(range(N_CORES)))
    outs = []
    for c in range(N_CORES):
        outT = res.results[c]["outT"]  # (256, 1024)
        outs.append(np.ascontiguousarray(outT.T))
    full = np.concatenate(outs, axis=0)  # (8192, 256)
    return full.reshape(B, T, NF).astype(np.float32)
